# revision 19
# baseline (speedup 1.0000x reference)
"""Dilated attention Trainium2 kernel.

Problem: B=4, H=16, T=8192, D=64, rates [1,2,3,4].
For rate r: segment S=2^(r+2), dilation dr=2^r; each head h attends causally
within segments l where l % dr == h % dr; output = mean over rates of the
scatter-added per-rate attention outputs.

Strategy (SPMD over 8 cores, 8 (b,h) pairs per core, all streams bf16):
  * Host pre-gathers each pair's selected segments per rate into compact
    sequences (7680 positions = 60 tiles of 128 per pair) and packs Q^T,K^T
    as one [128, 2, 3840] tensor: rows 0:64 hold even tiles, 64:128 odd
    tiles, loaded in a single ~1.9MB DMA per pair.
  * Compact half-tile scores: every segment length divides 64, so a tile's
    block-diagonal active region splits into (k,q < 64) and (k,q >= 64)
    halves. Quadrant matmuls (tile_position row=even/odd source rows,
    col=half) write each tile's scores^T into just 64 PSUM cols: low half on
    partitions 0:64, high on 64:128 -- halving exp/mask work. Even tiles land
    in PSUM bank A, odd tiles in bank B, so concurrent same-bank PE writers
    are always partition-disjoint col-quadrants (same-partition concurrent
    writes to one bank from different row strips lock up the device).
  * Per supergroup of 8 tiles: one ACT exp(0.125*s) over [128,512], one DVE
    mask multiply (compact 64-wide causal masks, broadcast-AP), then per 4
    tiles: 8 quadrant PV matmuls (lhsT=E half, rhs=[V half | 4.0]) into a
    bank-aligned [128,4,128] PSUM tile, and a PSUM->SBUF copy of
    [O | 4*denom] (alternating DVE/ACT) into a per-pair [128,60,65] output
    tile; finished slot ranges are flushed to DRAM in 4 chunked stores that
    overlap the remaining compute. (The 4.0 ones-column folds the mean over
    rates into the denominator; the host divides during the scatter-add.)
"""

import contextlib
import sys

import ml_dtypes
import numpy as np

try:
    import concourse.bass as bass  # noqa: F401
except ImportError:
    sys.path.insert(0, "/opt/trn_rl_repo")

import concourse.bass as bass
import concourse.mybir as mybir
import concourse.tile as tile
from concourse import bacc
from concourse.bass_utils import run_bass_kernel_spmd

B, H, T, D = 4, 16, 8192, 64
RATES = [1, 2, 3, 4]
N_CORES = 8
PAIRS_PER_CORE = (B * H) // N_CORES  # 8
TILE_Q = 128
GRP = 4  # tiles per PV/normalize group (PSUM bank limit)
SUPER_GRP = 8  # tiles per scores/exp/mask group
DV = D + 1  # 65: V plus ones column

SEGS = [2 ** (r + 2) for r in RATES]  # 8, 16, 32, 64
DILS = [2**r for r in RATES]  # 2, 4, 8, 16
TRS = [T // d for d in DILS]  # 4096, 2048, 1024, 512
G_TOTAL = sum(TRS)  # 7680
NTILES = [tr // TILE_Q for tr in TRS]  # 32, 16, 8, 4
N_TILES_TOT = G_TOTAL // TILE_Q  # 60
HALF = TILE_Q * N_TILES_TOT // 2  # 3840
HT = TILE_Q // 2  # 64: half-tile width (blocks never straddle it; S | 64)
QKV_W = 2 * HALF + N_TILES_TOT * DV  # 11580 bf16 elems/partition: qt|kt|v

MASK_ENG = "dve"  # "split" | "dve" | "gpsimd"
LOOKAHEAD = 3  # score supergroups in flight ahead of the consuming tail
STORE_ENG = "sync"  # "sync" | "gpsimd"
COPY_SPLIT = True  # alternate PSUM->SBUF copies between DVE and ACT
# after the Nth finished supergroup of a pair, flush these o slots
STORE_CHUNKS = {3: (0, 24), 5: (24, 40), 7: (40, 56), 8: (56, 60)}
LOAD_ENG = "sync"  # "sync" | "gpsimd"
QK_BUFS = 3
V_BUFS = 3
E_BUFS = 6
O_BUFS = 3
NP_BF16 = ml_dtypes.bfloat16
DT_DEV = mybir.dt.bfloat16  # SBUF/DRAM stream dtype
DT_PS = mybir.dt.float32  # PSUM accumulate dtype


def _slot_perm():
    """Natural slot order: compact half-tile PSUM packing needs no even/odd
    bank split, so slot == tile index."""
    return np.arange(N_TILES_TOT)


SLOT_PERM = _slot_perm()


def _build_masks() -> np.ndarray:
    """[128, 4 * 64] fp32 compact per-rate masks: rows 0:64 = mask[k, q] for
    the low 64x64 block; rows 64:128 repeat it (the block pattern is periodic
    with period S | 64, so low and high diagonal blocks are identical)."""
    m = np.zeros((TILE_Q, len(RATES) * HT), np.float32)
    k = np.arange(HT)[:, None]
    q = np.arange(HT)[None, :]
    for ri, s in enumerate(SEGS):
        allowed = ((q // s == k // s) & (k % s <= q % s)).astype(np.float32)
        m[0:HT, ri * HT : (ri + 1) * HT] = allowed
        m[HT:, ri * HT : (ri + 1) * HT] = allowed
    return m


def _sel_indices(h: int, r_idx: int) -> np.ndarray:
    s, dr = SEGS[r_idx], DILS[r_idx]
    lp = (T // s) // dr
    return (h % dr) + np.arange(lp) * dr


def _gather_pair(x: np.ndarray, h: int) -> np.ndarray:
    """x: [T, D] -> compact [7680, D] (concat of per-rate selected segments)."""
    parts = []
    for ri in range(len(RATES)):
        s = SEGS[ri]
        sel = _sel_indices(h, ri)
        parts.append(x.reshape(T // s, s, D)[sel].reshape(-1, D))
    return np.concatenate(parts, axis=0)


def _scatter_pair_h(og: np.ndarray, h: int) -> np.ndarray:
    out = np.zeros((T, D), np.float32)
    off = 0
    for ri in range(len(RATES)):
        s, tr = SEGS[ri], TRS[ri]
        sel = _sel_indices(h, ri)
        out.reshape(T // s, s, D)[sel] += og[off : off + tr].reshape(-1, s, D)
        off += tr
    return out


def _bcast_free(ap, count):
    """Repeat a [P, F] AP `count` times along a new middle free dim (step 0)."""
    return bass.AP(tensor=ap.tensor, offset=ap.offset,
                   ap=[ap.ap[0], [0, count], *ap.ap[1:]])


def _mask_engine(nc, gidx):
    if MASK_ENG == "dve":
        return nc.vector
    if MASK_ENG == "gpsimd":
        return nc.gpsimd
    return nc.gpsimd if gidx % 3 != 2 else nc.vector


def _group_list():
    """(ri, j0, sg) for each supergroup of one pair, in slot order."""
    out = []
    off_t = 0
    for ri in range(len(RATES)):
        n = NTILES[ri]
        sg = min(SUPER_GRP, n)
        for g in range(n // sg):
            out.append((ri, off_t + g * sg, sg))
        off_t += n
    return out


GROUPS = _group_list()


def _emit_head(nc, pools, qt_full, kt_full, gr):
    """Compact scores of one supergroup -> ps_s [128, sg*64].

    Tile t's block-diagonal-active region splits at the half boundary
    (S | 64): low block (k,q < 64) lands on psum partitions 0:64, high block
    (k,q >= 64) on partitions 64:128, both in cols [slot*64, slot*64+64).
    Even tiles live on qt/kt rows 0:64 (D contraction), odd on rows 64:128,
    so each even/odd pair fills all four PE quadrants concurrently."""
    dt = DT_PS
    ps_s_pool = pools[4]
    ri, j0, sg = gr
    sh = sg // 2
    m0 = j0 // 2
    # Two banks per supergroup, split by PE row strip: even tiles (qt/kt rows
    # 0:64) land in bank A cols [0, sh*64), odd tiles (rows 64:128) in bank B
    # cols [512, 512+sh*64). Concurrent same-bank writers are then always
    # partition-disjoint col-quadrants, never same-partition different-cols.
    ps_full = ps_s_pool.tile([TILE_Q, 2 * SUPER_GRP * HT], dt, tag="ps_s")
    for i in range(sh):
        mc = (m0 + i) * TILE_Q
        for par, r0 in ((0, 0), (1, 64)):  # even tile on rows 0:64, odd on 64:128
            c0 = par * (SUPER_GRP * HT) + i * HT
            nc.tensor.matmul(
                ps_full[0:64, c0 : c0 + HT],
                kt_full[r0 : r0 + 64, mc : mc + HT],
                qt_full[r0 : r0 + 64, mc : mc + HT],
                start=True,
                stop=True,
                tile_position=(r0, 0),
            )
            nc.tensor.matmul(
                ps_full[64:128, c0 : c0 + HT],
                kt_full[r0 : r0 + 64, mc + HT : mc + TILE_Q],
                qt_full[r0 : r0 + 64, mc + HT : mc + TILE_Q],
                start=True,
                stop=True,
                tile_position=(r0, 64),
            )
    return ps_full


STAGES = "all"  # "scores" | "exp" | "mask" | "pv" | "all"


def _emit_tail(nc, pools, ps_s, v_full, o_d, p, masks, gr, gidx, store=True,
               o_pair=None):
    """exp/mask/PV/copy of one supergroup; store once per pair."""
    dt = DT_DEV
    _, _, e_pool, o_pool, _, ps_o_pool = pools
    ri, j0, sg = gr
    sh = sg // 2
    if STAGES == "scores":
        return
    e = e_pool.tile([TILE_Q, sg * HT], dt, tag="e")
    em = e
    mask_sl = masks[:, ri * HT : (ri + 1) * HT]
    do_mask = STAGES in ("mask", "pv", "all")
    w = sh * HT
    ps_in = bass.AP(
        tensor=ps_s.tensor,
        offset=ps_s.offset,
        ap=[ps_s.ap[0], [SUPER_GRP * HT, 2], [1, w]],
    )
    nc.scalar.activation(
        e[:], ps_in, mybir.ActivationFunctionType.Exp, scale=0.125
    )
    if do_mask:
        _mask_engine(nc, gidx).tensor_mul(e[:], e[:], _bcast_free(mask_sl, sg))
    if STAGES in ("exp", "mask"):
        return

    o_sg = o_pair if o_pair is not None else o_pool.tile(
        [TILE_Q, N_TILES_TOT, DV], dt, tag="osg")
    for sub in range(sg // GRP):
        # GRP x 128-col fp32 sub-tiles: 512B stride keeps each 260B matmul
        # output inside a 512B-aligned region (no PSUM bank crossing), and the
        # whole tile is exactly one 2KB bank
        ps_o = ps_o_pool.tile([TILE_Q, GRP, TILE_Q], DT_PS, tag="ps_o")
        for i in range(GRP):
            s = sub * GRP + i
            # low half: queries 0:64 of the tile (k contraction on rows 0:64)
            ce = (s % 2) * (sh * HT) + (s // 2) * HT
            nc.tensor.matmul(
                ps_o[0:64, i, 0:DV],
                em[0:64, ce : ce + HT],
                v_full[0:64, j0 + s, :],
                start=True,
                stop=True,
                tile_position=(0, 0),
            )
            nc.tensor.matmul(
                ps_o[64:128, i, 0:DV],
                em[64:128, ce : ce + HT],
                v_full[64:128, j0 + s, :],
                start=True,
                stop=True,
                tile_position=(64, 64),
            )
        if STAGES == "pv":
            continue
        # raw [O | 4*denom] to SBUF; the host divides during the scatter-add
        dst = o_sg[:, j0 + sub * GRP : j0 + (sub + 1) * GRP, :]
        if COPY_SPLIT and (gidx * 2 + sub) % 3 == 2:
            nc.scalar.copy(out=dst, in_=ps_o[:, :, 0:DV])
        else:
            nc.vector.tensor_copy(out=dst, in_=ps_o[:, :, 0:DV])
    if STAGES == "pv":
        return
    if store:
        st = nc.gpsimd if STORE_ENG == "gpsimd" else nc.sync
        st.dma_start(out=o_d[p], in_=o_sg[:])


def _emit_body(nc, pools, qkv_d, o_d, masks):
    """Software-pipelined emission: scores of supergroup g+1 are emitted
    before the tail of supergroup g so the PE never waits on exp/mask."""
    dt = DT_DEV
    qk_pool, v_pool = pools[0], pools[1]
    from collections import deque

    o_pool = pools[3]
    gidx = 0
    o_tiles = {}
    grp_ctr = {}

    def _tail(pr):
        nonlocal gidx
        ps_s, v_full, p, gr = pr
        if p not in o_tiles:
            ot = o_pool.tile([TILE_Q, N_TILES_TOT, DV], dt, tag="osg")
            o_tiles[p] = ot
            grp_ctr[p] = 0
        _emit_tail(nc, pools, ps_s, v_full, o_d, p, masks, gr, gidx,
                   store=False, o_pair=o_tiles[p])
        g = grp_ctr[p] = grp_ctr[p] + 1
        chunk = STORE_CHUNKS.get(g)
        if chunk is not None:
            lo, hi = chunk
            nc.sync.dma_start(
                out=o_d[p, :, lo:hi, :], in_=o_tiles[p][:, lo:hi, :]
            )
        gidx += 1

    pending = deque()  # (ps_s, v_full, p, gr)
    for p in range(PAIRS_PER_CORE):
        qkv = qk_pool.tile([TILE_Q, QKV_W], dt, tag="qkv")
        qt_full = qkv[:, 0:HALF]
        kt_full = qkv[:, HALF : 2 * HALF]
        v_full = bass.AP(
            tensor=qkv.tensor,
            offset=qkv.offset + 2 * HALF,
            ap=[qkv.ap[0], [DV, N_TILES_TOT], [1, DV]],
        )
        ld = nc.gpsimd if LOAD_ENG == "gpsimd" else nc.sync
        ld.dma_start(out=qkv[:], in_=qkv_d[p])
        for gr in GROUPS:
            ps_s = _emit_head(nc, pools, qt_full, kt_full, gr)
            pending.append((ps_s, v_full, p, gr))
            if len(pending) > LOOKAHEAD:
                _tail(pending.popleft())
    while pending:
        _tail(pending.popleft())


def _emit_body_dma(nc, pools, qkv_d, o_d):
    """Same DRAM traffic as the real body, no compute."""
    dt = DT_DEV
    qk_pool, v_pool = pools[0], pools[1]
    for p in range(PAIRS_PER_CORE):
        qkv = qk_pool.tile([TILE_Q, QKV_W], dt, tag="qkv")
        nc.sync.dma_start(out=qkv[:], in_=qkv_d[p])
        nc.sync.dma_start(
            out=o_d[p],
            in_=bass.AP(
                tensor=qkv.tensor,
                offset=qkv.offset + 2 * HALF,
                ap=[qkv.ap[0], [DV, N_TILES_TOT], [1, DV]],
            ),
        )


def _emit_body_compute(nc, pools, qkv_d, o_d, masks):
    """Full compute on SBUF-resident data for one pair, repeated 8x."""
    dt = DT_DEV
    qk_pool, v_pool = pools[0], pools[1]
    qkv = qk_pool.tile([TILE_Q, QKV_W], dt, tag="qkv")
    qt_full = qkv[:, 0:HALF]
    kt_full = qkv[:, HALF : 2 * HALF]
    v_full = bass.AP(
        tensor=qkv.tensor,
        offset=qkv.offset + 2 * HALF,
        ap=[qkv.ap[0], [DV, N_TILES_TOT], [1, DV]],
    )
    nc.sync.dma_start(out=qkv[:], in_=qkv_d[0])
    gidx = 0
    prev = None
    for p in range(PAIRS_PER_CORE):
        for gr in GROUPS:
            ps_s = _emit_head(nc, pools, qt_full, kt_full, gr)
            if prev is not None:
                _emit_tail(nc, pools, prev[0], v_full, o_d, 0, masks, prev[1],
                           gidx, store=False)
                gidx += 1
            prev = (ps_s, gr)
    _emit_tail(nc, pools, prev[0], v_full, o_d, 0, masks, prev[1], gidx, store=True)


def _build_program(body_reps: int = 1, variant: str = "full"):
    nc = bacc.Bacc(None, target_bir_lowering=False, debug=False)
    dt = DT_DEV
    qkv_d = nc.declare_dram_parameter(
        "qkv", [PAIRS_PER_CORE, TILE_Q, QKV_W], dt, isOutput=False
    )
    m_d = nc.declare_dram_parameter(
        "mask", [TILE_Q, len(RATES) * HT], dt, isOutput=False
    )
    o_d = nc.declare_dram_parameter(
        "o", [PAIRS_PER_CORE, TILE_Q, N_TILES_TOT, DV], dt, isOutput=True
    )

    with tile.TileContext(nc) as tc:
        with (
            tc.tile_pool(name="consts", bufs=1) as consts,
            tc.tile_pool(name="qk", bufs=QK_BUFS) as qk_pool,
            tc.tile_pool(name="vt", bufs=1) as v_pool,
            tc.tile_pool(name="ew", bufs=E_BUFS) as e_pool,
            tc.tile_pool(name="ot", bufs=O_BUFS) as o_pool,
            tc.tile_pool(name="ps_s", bufs=2, space="PSUM") as ps_s_pool,
            tc.tile_pool(name="ps_o", bufs=4, space="PSUM") as ps_o_pool,
        ):
            masks = consts.tile([TILE_Q, len(RATES) * HT], dt)
            nc.sync.dma_start(out=masks[:], in_=m_d[:])
            pools = (qk_pool, v_pool, e_pool, o_pool, ps_s_pool, ps_o_pool)

            loop_ctx = (
                tc.For_i(0, body_reps, 1) if body_reps > 1 else contextlib.nullcontext()
            )
            with loop_ctx:
                if variant == "full":
                    _emit_body(nc, pools, qkv_d, o_d, masks)
                elif variant == "dma":
                    _emit_body_dma(nc, pools, qkv_d, o_d)
                elif variant == "compute":
                    _emit_body_compute(nc, pools, qkv_d, o_d, masks)
    nc.compile()
    return nc


_PROGRAM_CACHE = {}


def _get_program():
    if "nc" not in _PROGRAM_CACHE:
        _PROGRAM_CACHE["nc"] = _build_program()
    return _PROGRAM_CACHE["nc"]


def prepare_inputs(Q, K, V):
    """Host-side shard+gather+transpose+pack. Returns per-core input maps."""
    Q = np.asarray(Q, dtype=np.float32)
    K = np.asarray(K, dtype=np.float32)
    V = np.asarray(V, dtype=np.float32)
    masks = _build_masks()
    in_maps = []
    for c in range(N_CORES):
        qkv = np.empty((PAIRS_PER_CORE, TILE_Q, QKV_W), NP_BF16)
        qt = qkv[:, :, 0:HALF]
        kt = qkv[:, :, HALF : 2 * HALF]
        vg = qkv[:, :, 2 * HALF :].reshape(
            PAIRS_PER_CORE, TILE_Q, N_TILES_TOT, DV
        )
        vg[:, :, :, D] = 4.0
        for p in range(PAIRS_PER_CORE):
            pair = c * PAIRS_PER_CORE + p
            b, h = divmod(pair, H)
            qg = _gather_pair(Q[b, h], h).T  # [64, 7680]
            kg = _gather_pair(K[b, h], h).T
            # pack: col-block m rows 0:64 = tile 2m, rows 64:128 = tile 2m+1
            qt[p, 0:64] = qg.reshape(D, N_TILES_TOT, TILE_Q)[:, 0::2].reshape(D, HALF)
            qt[p, 64:128] = qg.reshape(D, N_TILES_TOT, TILE_Q)[:, 1::2].reshape(D, HALF)
            kt[p, 0:64] = kg.reshape(D, N_TILES_TOT, TILE_Q)[:, 0::2].reshape(D, HALF)
            kt[p, 64:128] = kg.reshape(D, N_TILES_TOT, TILE_Q)[:, 1::2].reshape(D, HALF)
            vp = _gather_pair(V[b, h], h)  # [7680, 64]
            vt = vp.reshape(N_TILES_TOT, TILE_Q, D)[SLOT_PERM]  # slot order
            vg[p, :, :, 0:D] = vt.transpose(1, 0, 2)
        in_maps.append({"qkv": qkv, "mask": masks.astype(NP_BF16)})
    return in_maps


def finish_outputs(results):
    """results: list of per-core {'o': [8, 128, 60, 64]} -> full [B, H, T, D]."""
    inv = np.argsort(SLOT_PERM)
    out = np.zeros((B, H, T, D), np.float32)
    for c in range(N_CORES):
        og = np.asarray(results[c]["o"]).astype(np.float32)  # [8, 128, 60, 64]
        for p in range(PAIRS_PER_CORE):
            pair = c * PAIRS_PER_CORE + p
            b, h = divmod(pair, H)
            tiles = og[p].transpose(1, 0, 2)  # [60(slots), 128, 65]
            compact = tiles[inv].reshape(G_TOTAL, DV)
            out[b, h] = _scatter_pair_h(compact[:, :D] / compact[:, D:], h)
    return out


def kernel(Q, K, V):
    nc = _get_program()
    in_maps = prepare_inputs(Q, K, V)
    res = run_bass_kernel_spmd(nc, in_maps, list(range(N_CORES)))
    return finish_outputs(res.results)



# revision 20
# speedup vs baseline: 1.0345x; 1.0345x over previous
"""Dilated attention Trainium2 kernel.

Problem: B=4, H=16, T=8192, D=64, rates [1,2,3,4].
For rate r: segment S=2^(r+2), dilation dr=2^r; each head h attends causally
within segments l where l % dr == h % dr; output = mean over rates of the
scatter-added per-rate attention outputs.

Strategy (SPMD over 8 cores, 8 (b,h) pairs per core, all streams bf16):
  * Host pre-gathers each pair's selected segments per rate into compact
    sequences (7680 positions = 60 tiles of 128 per pair) and packs Q^T,K^T
    as one [128, 2, 3840] tensor: rows 0:64 hold even tiles, 64:128 odd
    tiles, loaded in a single ~1.9MB DMA per pair.
  * Compact half-tile scores: every segment length divides 64, so a tile's
    block-diagonal active region splits into (k,q < 64) and (k,q >= 64)
    halves. Quadrant matmuls (tile_position row=even/odd source rows,
    col=half) write each tile's scores^T into just 64 PSUM cols: low half on
    partitions 0:64, high on 64:128 -- halving exp/mask work. Even tiles land
    in PSUM bank A, odd tiles in bank B, so concurrent same-bank PE writers
    are always partition-disjoint col-quadrants (same-partition concurrent
    writes to one bank from different row strips lock up the device).
  * Per supergroup of 8 tiles: one ACT exp(0.125*s) over [128,512], one DVE
    mask multiply (compact 64-wide causal masks, broadcast-AP), then per 4
    tiles: 8 quadrant PV matmuls (lhsT=E half, rhs=[V half | 4.0]) into a
    bank-aligned [128,4,128] PSUM tile, and a PSUM->SBUF copy of
    [O | 4*denom] (alternating DVE/ACT) into a per-pair [128,60,65] output
    tile; finished slot ranges are flushed to DRAM in 4 chunked stores that
    overlap the remaining compute. (The 4.0 ones-column folds the mean over
    rates into the denominator; the host divides during the scatter-add.)
"""

import contextlib
import sys

import ml_dtypes
import numpy as np

try:
    import concourse.bass as bass  # noqa: F401
except ImportError:
    sys.path.insert(0, "/opt/trn_rl_repo")

import concourse.bass as bass
import concourse.mybir as mybir
import concourse.tile as tile
from concourse import bacc
from concourse.bass_utils import run_bass_kernel_spmd

B, H, T, D = 4, 16, 8192, 64
RATES = [1, 2, 3, 4]
N_CORES = 8
PAIRS_PER_CORE = (B * H) // N_CORES  # 8
TILE_Q = 128
GRP = 4  # tiles per PV/normalize group (PSUM bank limit)
SUPER_GRP = 8  # tiles per scores/exp/mask group
DV = D + 1  # 65: V plus ones column

SEGS = [2 ** (r + 2) for r in RATES]  # 8, 16, 32, 64
DILS = [2**r for r in RATES]  # 2, 4, 8, 16
TRS = [T // d for d in DILS]  # 4096, 2048, 1024, 512
G_TOTAL = sum(TRS)  # 7680
NTILES = [tr // TILE_Q for tr in TRS]  # 32, 16, 8, 4
N_TILES_TOT = G_TOTAL // TILE_Q  # 60
HALF = TILE_Q * N_TILES_TOT // 2  # 3840
HT = TILE_Q // 2  # 64: half-tile width (blocks never straddle it; S | 64)
QKV_W = 2 * HALF + N_TILES_TOT * DV  # 11580 bf16 elems/partition: qt|kt|v

MASK_ENG = "dve"  # "split" | "dve" | "gpsimd"
LOOKAHEAD = 3  # score supergroups in flight ahead of the consuming tail
STORE_ENG = "gpsimd"  # "sync" | "gpsimd"
COPY_SPLIT = True  # alternate PSUM->SBUF copies between DVE and ACT
# after the Nth finished supergroup of a pair, flush these o slots
STORE_CHUNKS = {3: (0, 24), 5: (24, 40), 7: (40, 56), 8: (56, 60)}
LOAD_ENG = "sync"  # "sync" | "gpsimd"
QK_BUFS = 3
V_BUFS = 3
E_BUFS = 6
O_BUFS = 3
NP_BF16 = ml_dtypes.bfloat16
DT_DEV = mybir.dt.bfloat16  # SBUF/DRAM stream dtype
DT_PS = mybir.dt.float32  # PSUM accumulate dtype


def _slot_perm():
    """Natural slot order: compact half-tile PSUM packing needs no even/odd
    bank split, so slot == tile index."""
    return np.arange(N_TILES_TOT)


SLOT_PERM = _slot_perm()


def _build_masks() -> np.ndarray:
    """[128, 4 * 64] fp32 compact per-rate masks: rows 0:64 = mask[k, q] for
    the low 64x64 block; rows 64:128 repeat it (the block pattern is periodic
    with period S | 64, so low and high diagonal blocks are identical)."""
    m = np.zeros((TILE_Q, len(RATES) * HT), np.float32)
    k = np.arange(HT)[:, None]
    q = np.arange(HT)[None, :]
    for ri, s in enumerate(SEGS):
        allowed = ((q // s == k // s) & (k % s <= q % s)).astype(np.float32)
        m[0:HT, ri * HT : (ri + 1) * HT] = allowed
        m[HT:, ri * HT : (ri + 1) * HT] = allowed
    return m


def _sel_indices(h: int, r_idx: int) -> np.ndarray:
    s, dr = SEGS[r_idx], DILS[r_idx]
    lp = (T // s) // dr
    return (h % dr) + np.arange(lp) * dr


def _gather_pair(x: np.ndarray, h: int) -> np.ndarray:
    """x: [T, D] -> compact [7680, D] (concat of per-rate selected segments)."""
    parts = []
    for ri in range(len(RATES)):
        s = SEGS[ri]
        sel = _sel_indices(h, ri)
        parts.append(x.reshape(T // s, s, D)[sel].reshape(-1, D))
    return np.concatenate(parts, axis=0)


def _scatter_pair_h(og: np.ndarray, h: int) -> np.ndarray:
    out = np.zeros((T, D), np.float32)
    off = 0
    for ri in range(len(RATES)):
        s, tr = SEGS[ri], TRS[ri]
        sel = _sel_indices(h, ri)
        out.reshape(T // s, s, D)[sel] += og[off : off + tr].reshape(-1, s, D)
        off += tr
    return out


def _bcast_free(ap, count):
    """Repeat a [P, F] AP `count` times along a new middle free dim (step 0)."""
    return bass.AP(tensor=ap.tensor, offset=ap.offset,
                   ap=[ap.ap[0], [0, count], *ap.ap[1:]])


def _mask_engine(nc, gidx):
    if MASK_ENG == "dve":
        return nc.vector
    if MASK_ENG == "gpsimd":
        return nc.gpsimd
    return nc.gpsimd if gidx % 3 != 2 else nc.vector


def _group_list():
    """(ri, j0, sg) for each supergroup of one pair, in slot order."""
    out = []
    off_t = 0
    for ri in range(len(RATES)):
        n = NTILES[ri]
        sg = min(SUPER_GRP, n)
        for g in range(n // sg):
            out.append((ri, off_t + g * sg, sg))
        off_t += n
    return out


GROUPS = _group_list()


def _emit_head(nc, pools, qt_full, kt_full, gr):
    """Compact scores of one supergroup -> ps_s [128, sg*64].

    Tile t's block-diagonal-active region splits at the half boundary
    (S | 64): low block (k,q < 64) lands on psum partitions 0:64, high block
    (k,q >= 64) on partitions 64:128, both in cols [slot*64, slot*64+64).
    Even tiles live on qt/kt rows 0:64 (D contraction), odd on rows 64:128,
    so each even/odd pair fills all four PE quadrants concurrently."""
    dt = DT_PS
    ps_s_pool = pools[4]
    ri, j0, sg = gr
    sh = sg // 2
    m0 = j0 // 2
    # Two banks per supergroup, split by PE row strip: even tiles (qt/kt rows
    # 0:64) land in bank A cols [0, sh*64), odd tiles (rows 64:128) in bank B
    # cols [512, 512+sh*64). Concurrent same-bank writers are then always
    # partition-disjoint col-quadrants, never same-partition different-cols.
    ps_full = ps_s_pool.tile([TILE_Q, 2 * SUPER_GRP * HT], dt, tag="ps_s")
    for i in range(sh):
        mc = (m0 + i) * TILE_Q
        for par, r0 in ((0, 0), (1, 64)):  # even tile on rows 0:64, odd on 64:128
            c0 = par * (SUPER_GRP * HT) + i * HT
            nc.tensor.matmul(
                ps_full[0:64, c0 : c0 + HT],
                kt_full[r0 : r0 + 64, mc : mc + HT],
                qt_full[r0 : r0 + 64, mc : mc + HT],
                start=True,
                stop=True,
                tile_position=(r0, 0),
            )
            nc.tensor.matmul(
                ps_full[64:128, c0 : c0 + HT],
                kt_full[r0 : r0 + 64, mc + HT : mc + TILE_Q],
                qt_full[r0 : r0 + 64, mc + HT : mc + TILE_Q],
                start=True,
                stop=True,
                tile_position=(r0, 64),
            )
    return ps_full


STAGES = "all"  # "scores" | "exp" | "mask" | "pv" | "all"


def _emit_tail(nc, pools, ps_s, v_full, o_d, p, masks, gr, gidx, store=True,
               o_pair=None):
    """exp/mask/PV/copy of one supergroup; store once per pair."""
    dt = DT_DEV
    _, _, e_pool, o_pool, _, ps_o_pool = pools
    ri, j0, sg = gr
    sh = sg // 2
    if STAGES == "scores":
        return
    e = e_pool.tile([TILE_Q, sg * HT], dt, tag="e")
    em = e
    mask_sl = masks[:, ri * HT : (ri + 1) * HT]
    do_mask = STAGES in ("mask", "pv", "all")
    w = sh * HT
    ps_in = bass.AP(
        tensor=ps_s.tensor,
        offset=ps_s.offset,
        ap=[ps_s.ap[0], [SUPER_GRP * HT, 2], [1, w]],
    )
    nc.scalar.activation(
        e[:], ps_in, mybir.ActivationFunctionType.Exp, scale=0.125
    )
    if do_mask:
        _mask_engine(nc, gidx).tensor_mul(e[:], e[:], _bcast_free(mask_sl, sg))
    if STAGES in ("exp", "mask"):
        return

    o_sg = o_pair if o_pair is not None else o_pool.tile(
        [TILE_Q, N_TILES_TOT, DV], dt, tag="osg")
    for sub in range(sg // GRP):
        # GRP x 128-col fp32 sub-tiles: 512B stride keeps each 260B matmul
        # output inside a 512B-aligned region (no PSUM bank crossing), and the
        # whole tile is exactly one 2KB bank
        ps_o = ps_o_pool.tile([TILE_Q, GRP, TILE_Q], DT_PS, tag="ps_o")
        for i in range(GRP):
            s = sub * GRP + i
            # low half: queries 0:64 of the tile (k contraction on rows 0:64)
            ce = (s % 2) * (sh * HT) + (s // 2) * HT
            nc.tensor.matmul(
                ps_o[0:64, i, 0:DV],
                em[0:64, ce : ce + HT],
                v_full[0:64, j0 + s, :],
                start=True,
                stop=True,
                tile_position=(0, 0),
            )
            nc.tensor.matmul(
                ps_o[64:128, i, 0:DV],
                em[64:128, ce : ce + HT],
                v_full[64:128, j0 + s, :],
                start=True,
                stop=True,
                tile_position=(64, 64),
            )
        if STAGES == "pv":
            continue
        # raw [O | 4*denom] to SBUF; the host divides during the scatter-add
        dst = o_sg[:, j0 + sub * GRP : j0 + (sub + 1) * GRP, :]
        if COPY_SPLIT and (gidx * 2 + sub) % 3 == 2:
            nc.scalar.copy(out=dst, in_=ps_o[:, :, 0:DV])
        else:
            nc.vector.tensor_copy(out=dst, in_=ps_o[:, :, 0:DV])
    if STAGES == "pv":
        return
    if store:
        st = nc.gpsimd if STORE_ENG == "gpsimd" else nc.sync
        st.dma_start(out=o_d[p], in_=o_sg[:])


def _emit_body(nc, pools, qkv_d, o_d, masks):
    """Software-pipelined emission: scores of supergroup g+1 are emitted
    before the tail of supergroup g so the PE never waits on exp/mask."""
    dt = DT_DEV
    qk_pool, v_pool = pools[0], pools[1]
    from collections import deque

    o_pool = pools[3]
    gidx = 0
    o_tiles = {}
    grp_ctr = {}

    def _tail(pr):
        nonlocal gidx
        ps_s, v_full, p, gr = pr
        if p not in o_tiles:
            ot = o_pool.tile([TILE_Q, N_TILES_TOT, DV], dt, tag="osg")
            o_tiles[p] = ot
            grp_ctr[p] = 0
        _emit_tail(nc, pools, ps_s, v_full, o_d, p, masks, gr, gidx,
                   store=False, o_pair=o_tiles[p])
        g = grp_ctr[p] = grp_ctr[p] + 1
        chunk = STORE_CHUNKS.get(g)
        if chunk is not None:
            lo, hi = chunk
            st = nc.gpsimd if STORE_ENG == "gpsimd" else nc.sync
            st.dma_start(
                out=o_d[p, :, lo:hi, :], in_=o_tiles[p][:, lo:hi, :]
            )
        gidx += 1

    pending = deque()  # (ps_s, v_full, p, gr)
    for p in range(PAIRS_PER_CORE):
        qkv = qk_pool.tile([TILE_Q, QKV_W], dt, tag="qkv")
        qt_full = qkv[:, 0:HALF]
        kt_full = qkv[:, HALF : 2 * HALF]
        v_full = bass.AP(
            tensor=qkv.tensor,
            offset=qkv.offset + 2 * HALF,
            ap=[qkv.ap[0], [DV, N_TILES_TOT], [1, DV]],
        )
        ld = nc.gpsimd if LOAD_ENG == "gpsimd" else nc.sync
        ld.dma_start(out=qkv[:], in_=qkv_d[p])
        for gr in GROUPS:
            ps_s = _emit_head(nc, pools, qt_full, kt_full, gr)
            pending.append((ps_s, v_full, p, gr))
            if len(pending) > LOOKAHEAD:
                _tail(pending.popleft())
    while pending:
        _tail(pending.popleft())


def _emit_body_dma(nc, pools, qkv_d, o_d):
    """Same DRAM traffic as the real body, no compute."""
    dt = DT_DEV
    qk_pool, v_pool = pools[0], pools[1]
    for p in range(PAIRS_PER_CORE):
        qkv = qk_pool.tile([TILE_Q, QKV_W], dt, tag="qkv")
        nc.sync.dma_start(out=qkv[:], in_=qkv_d[p])
        nc.sync.dma_start(
            out=o_d[p],
            in_=bass.AP(
                tensor=qkv.tensor,
                offset=qkv.offset + 2 * HALF,
                ap=[qkv.ap[0], [DV, N_TILES_TOT], [1, DV]],
            ),
        )


def _emit_body_compute(nc, pools, qkv_d, o_d, masks):
    """Full compute on SBUF-resident data for one pair, repeated 8x."""
    dt = DT_DEV
    qk_pool, v_pool = pools[0], pools[1]
    qkv = qk_pool.tile([TILE_Q, QKV_W], dt, tag="qkv")
    qt_full = qkv[:, 0:HALF]
    kt_full = qkv[:, HALF : 2 * HALF]
    v_full = bass.AP(
        tensor=qkv.tensor,
        offset=qkv.offset + 2 * HALF,
        ap=[qkv.ap[0], [DV, N_TILES_TOT], [1, DV]],
    )
    nc.sync.dma_start(out=qkv[:], in_=qkv_d[0])
    gidx = 0
    prev = None
    for p in range(PAIRS_PER_CORE):
        for gr in GROUPS:
            ps_s = _emit_head(nc, pools, qt_full, kt_full, gr)
            if prev is not None:
                _emit_tail(nc, pools, prev[0], v_full, o_d, 0, masks, prev[1],
                           gidx, store=False)
                gidx += 1
            prev = (ps_s, gr)
    _emit_tail(nc, pools, prev[0], v_full, o_d, 0, masks, prev[1], gidx, store=True)


def _build_program(body_reps: int = 1, variant: str = "full"):
    nc = bacc.Bacc(None, target_bir_lowering=False, debug=False)
    dt = DT_DEV
    qkv_d = nc.declare_dram_parameter(
        "qkv", [PAIRS_PER_CORE, TILE_Q, QKV_W], dt, isOutput=False
    )
    m_d = nc.declare_dram_parameter(
        "mask", [TILE_Q, len(RATES) * HT], dt, isOutput=False
    )
    o_d = nc.declare_dram_parameter(
        "o", [PAIRS_PER_CORE, TILE_Q, N_TILES_TOT, DV], dt, isOutput=True
    )

    with tile.TileContext(nc) as tc:
        with (
            tc.tile_pool(name="consts", bufs=1) as consts,
            tc.tile_pool(name="qk", bufs=QK_BUFS) as qk_pool,
            tc.tile_pool(name="vt", bufs=1) as v_pool,
            tc.tile_pool(name="ew", bufs=E_BUFS) as e_pool,
            tc.tile_pool(name="ot", bufs=O_BUFS) as o_pool,
            tc.tile_pool(name="ps_s", bufs=2, space="PSUM") as ps_s_pool,
            tc.tile_pool(name="ps_o", bufs=4, space="PSUM") as ps_o_pool,
        ):
            masks = consts.tile([TILE_Q, len(RATES) * HT], dt)
            nc.sync.dma_start(out=masks[:], in_=m_d[:])
            pools = (qk_pool, v_pool, e_pool, o_pool, ps_s_pool, ps_o_pool)

            loop_ctx = (
                tc.For_i(0, body_reps, 1) if body_reps > 1 else contextlib.nullcontext()
            )
            with loop_ctx:
                if variant == "full":
                    _emit_body(nc, pools, qkv_d, o_d, masks)
                elif variant == "dma":
                    _emit_body_dma(nc, pools, qkv_d, o_d)
                elif variant == "compute":
                    _emit_body_compute(nc, pools, qkv_d, o_d, masks)
    nc.compile()
    return nc


_PROGRAM_CACHE = {}


def _get_program():
    if "nc" not in _PROGRAM_CACHE:
        _PROGRAM_CACHE["nc"] = _build_program()
    return _PROGRAM_CACHE["nc"]


def prepare_inputs(Q, K, V):
    """Host-side shard+gather+transpose+pack. Returns per-core input maps."""
    Q = np.asarray(Q, dtype=np.float32)
    K = np.asarray(K, dtype=np.float32)
    V = np.asarray(V, dtype=np.float32)
    masks = _build_masks()
    in_maps = []
    for c in range(N_CORES):
        qkv = np.empty((PAIRS_PER_CORE, TILE_Q, QKV_W), NP_BF16)
        qt = qkv[:, :, 0:HALF]
        kt = qkv[:, :, HALF : 2 * HALF]
        vg = qkv[:, :, 2 * HALF :].reshape(
            PAIRS_PER_CORE, TILE_Q, N_TILES_TOT, DV
        )
        vg[:, :, :, D] = 4.0
        for p in range(PAIRS_PER_CORE):
            pair = c * PAIRS_PER_CORE + p
            b, h = divmod(pair, H)
            qg = _gather_pair(Q[b, h], h).T  # [64, 7680]
            kg = _gather_pair(K[b, h], h).T
            # pack: col-block m rows 0:64 = tile 2m, rows 64:128 = tile 2m+1
            qt[p, 0:64] = qg.reshape(D, N_TILES_TOT, TILE_Q)[:, 0::2].reshape(D, HALF)
            qt[p, 64:128] = qg.reshape(D, N_TILES_TOT, TILE_Q)[:, 1::2].reshape(D, HALF)
            kt[p, 0:64] = kg.reshape(D, N_TILES_TOT, TILE_Q)[:, 0::2].reshape(D, HALF)
            kt[p, 64:128] = kg.reshape(D, N_TILES_TOT, TILE_Q)[:, 1::2].reshape(D, HALF)
            vp = _gather_pair(V[b, h], h)  # [7680, 64]
            vt = vp.reshape(N_TILES_TOT, TILE_Q, D)[SLOT_PERM]  # slot order
            vg[p, :, :, 0:D] = vt.transpose(1, 0, 2)
        in_maps.append({"qkv": qkv, "mask": masks.astype(NP_BF16)})
    return in_maps


def finish_outputs(results):
    """results: list of per-core {'o': [8, 128, 60, 64]} -> full [B, H, T, D]."""
    inv = np.argsort(SLOT_PERM)
    out = np.zeros((B, H, T, D), np.float32)
    for c in range(N_CORES):
        og = np.asarray(results[c]["o"]).astype(np.float32)  # [8, 128, 60, 64]
        for p in range(PAIRS_PER_CORE):
            pair = c * PAIRS_PER_CORE + p
            b, h = divmod(pair, H)
            tiles = og[p].transpose(1, 0, 2)  # [60(slots), 128, 65]
            compact = tiles[inv].reshape(G_TOTAL, DV)
            out[b, h] = _scatter_pair_h(compact[:, :D] / compact[:, D:], h)
    return out


def kernel(Q, K, V):
    nc = _get_program()
    in_maps = prepare_inputs(Q, K, V)
    res = run_bass_kernel_spmd(nc, in_maps, list(range(N_CORES)))
    return finish_outputs(res.results)

